# revision 30
# baseline (speedup 1.0000x reference)
"""Trainium2 Bass kernel for nn_BoundMemUpdate (spiking membrane update).

Computes, for x:[T,B,D], W:[D,D], b:[D]:
    mm[t] = x[t] @ W.T + b
    m[t] = mm[t] + m[t-1] * (1 - s[t-1]) * 0.5
    s[t] = (m[t] >= 1.0)
Returns (m, s), each [T, B, D] float32.

Sharding: output-dim (D_out) sharded 8 ways across cores (512 each);
x replicated, W/b sharded by rows. The recurrence is per-neuron
elementwise, so no cross-core communication is needed.

Matmul: single-term fp16. x and W are rounded to fp16; the PE forms
exact fp16 products with fp32 accumulation, so the only error is the
input rounding (~2^-11 relative per operand). On the fixed seed-0
problem instance this measures 6.5e-3 concatenated relative error
(506 spike flips of 8.4M) against the fp32 reference -- 3x inside
the 2e-2 gate. W is the PE-stationary operand and x the moving one,
giving output tiles [o_part, (t,b)] and 512 matmuls of 518 cycles
per core (~111 us of PE streaming at 2.4 GHz).

Schedule: 4 passes over t-pairs, 4 o-block PSUM chains per pass with
K=4096 contraction. The vector engine fuses the drain with the
temporal recurrence (scalar_tensor_tensor / tensor_scalar with a
per-partition bias AP -- no scalar-engine ACTIVATE, so the Act ring
carries only DMA traffic and skips its table load). x is stored
pass-interleaved in DRAM ([pass][128][kt][t][b]) so every DMA is
contiguous on both sides. Pass 0 streams k-outer so the PE consumes
(W, x) chunks as they land. The NEFF preamble blocks all DMA for the
first ~7us and the first chunk completion lands ~4.5us after the
queues open, so a 12-matmul junk chain (no DMA dependencies) ramps
the HAM clock gate to 2.4 GHz during exactly that dead window and
hands off to the real k-outer stream as the first (W, x) chunks
arrive; 512 KB chunks keep the early completion cadence (~2.5-3us
per chunk, latency-floored) ahead of warm consumption (3.5us per
4-kt chunk). The last pass splits each o-block chain into per-t
half-bank chains (all t=2p chains before any t=2p+1 chain) so the
expensive t=6 drain overlaps t=7 matmuls and only one short drain
(fp16 add straight into the output tile, m on the SP ring / s on the
Act ring in parallel) remains after the final matmul. Outputs go out
as m:fp16 / s:fp8 in [t, o, b] layout; the host widens and
transposes during the final gather.
"""
import os
import numpy as np

import concourse.bass as bass
import concourse.mybir as mybir
from concourse import bacc
from concourse.tile import TileContext
from concourse.bass_utils import run_bass_kernel_spmd

T, B, D = 8, 256, 4096
N_CORES = 8
O_SHARD = D // N_CORES   # 512
KT = D // 128            # 32 fp16 k-tiles
OB = O_SHARD // 128      # 4 output blocks
NPASS = T // 2           # 4 t-pair passes
ALPHA = 0.5
M_TH = 1.0
WARM_MM = int(os.environ.get("BMU_WARM", "12"))
PSB = int(os.environ.get("BMU_PSB", "1"))       # psum pool bufs
XCH = int(os.environ.get("BMU_XCH", "8"))       # x DMA chunks per pass
WCH = int(os.environ.get("BMU_WCH", "8"))       # W DMA chunks
# pass-0 chunk sizes in kt units (each kt = 512 cols of both W and x).
# 512 KB chunks: early DMA completions land at a latency-floored
# ~2.5-3us cadence regardless of size, so chunks must carry >=4 kt to
# outpace warm PE consumption (0.864us/kt); fewer, bigger chunks also
# keep the issue cost (~0.6us of engine time per dma_start, <=4 in
# flight per ring) low.
CH0 = [int(v) for v in
       os.environ.get("BMU_CH0", "5,4,4,4,4,4,4,3").split(",")]
assert sum(CH0) == KT
OFFLOAD = os.environ.get("BMU_OFFLOAD", "0") == "1"  # s8->gpsimd, m16->scalar
TISPLIT = os.environ.get("BMU_TISPLIT", "1") == "1"  # last pass: per-t chains

F16 = mybir.dt.float16
F8 = mybir.dt.float8e4
F32 = mybir.dt.float32
NP_F16 = np.float16

_cache = {}


def _build_kernel(reps: int = 1):
    nc = bacc.Bacc("TRN2", target_bir_lowering=False, debug=False,
                   num_devices=N_CORES)

    NF16 = KT * 2 * B        # per-pass x free size (16384)

    wh16_d = nc.dram_tensor("wh16", [128, KT * O_SHARD], F16,
                            kind="ExternalInput").ap()
    xh16_d = nc.dram_tensor("xh16", [NPASS, 128, NF16], F16,
                            kind="ExternalInput").ap()
    bcol_d = nc.dram_tensor("bcol", [128, OB], F32,
                            kind="ExternalInput").ap()
    m_d = nc.dram_tensor("m_out", [T, O_SHARD, B], F16,
                         kind="ExternalOutput").ap()
    s_d = nc.dram_tensor("s_out", [T, O_SHARD, B], F8,
                         kind="ExternalOutput").ap()

    with TileContext(nc) as tc:
        with tc.tile_pool(name="wpool", bufs=1) as wpool, \
             tc.tile_pool(name="xhpool", bufs=2) as xhpool, \
             tc.tile_pool(name="cpool", bufs=1) as cpool, \
             tc.tile_pool(name="mpool", bufs=4) as mpool, \
             tc.tile_pool(name="opool", bufs=4) as opool, \
             tc.tile_pool(name="spool", bufs=4) as spool, \
             tc.tile_pool(name="upool", bufs=3) as upool, \
             tc.tile_pool(name="psum", bufs=PSB, space="PSUM") as psum_pool:

            whs = wpool.tile([128, KT * O_SHARD], F16, name="whs")
            bcol_t = cpool.tile([128, OB], F32)
            d_t = [cpool.tile([128, B], F32, name=f"d{ob}") for ob in range(OB)]
            warm_t = cpool.tile([128, 2 * B], F16, name="warm")
            dump_t = cpool.tile([128, 16], F32, name="dump")

            xsz = NF16 // XCH
            wsz = KT * O_SHARD // WCH

            # pass-0 load, interleaved in PE consumption order (k-outer
            # pass 0 walks kt 0..31 across o-blocks, so pair (wh16, xh16)
            # chunks by kt range); the junk warmup chain covers the
            # ~4.5us from queue-open to the first chunk completion.
            xh0 = xhpool.tile([128, NF16], F16, tag="xh")
            # W rides the Activation-engine DGE queue, x the SP queue --
            # two hardware rings drain the cold start in parallel. The
            # first W/x chunks gate the first matmul, so they issue
            # before anything else; bcol (needed only at the first
            # drain, ~15us later) rides behind the first W chunk.
            off = 0
            for ci, ckt in enumerate(CH0):
                sl = slice(off * 512, (off + ckt) * 512)
                nc.scalar.dma_start(out=whs[:, sl], in_=wh16_d[:, sl])
                nc.sync.dma_start(out=xh0[:, sl], in_=xh16_d[0][:, sl])
                if ci == 0:
                    nc.scalar.dma_start(out=bcol_t, in_=bcol_d)
                off += ckt

            wh_k = whs.rearrange("p (kt o) -> p kt o", kt=KT)

            def warmup():
                if WARM_MM <= 0:
                    return
                # vector memset: ready well inside the NEFF preamble, so
                # the junk chain starts the instant the PE queue opens
                nc.vector.memset(warm_t, 0.0)
                # dedicated bank: no real chain ever waits on the
                # warmup drain (5 of 8 PSUM banks in use)
                wp = psum_pool.tile([128, 2 * B], F32, tag="wps",
                                    name="warmpsum")
                for i in range(WARM_MM):
                    nc.tensor.matmul(wp, warm_t[:, :128], warm_t,
                                     start=(i == 0), stop=(i == WARM_MM - 1))
                nc.vector.tensor_copy(out=dump_t, in_=wp[:, :16])

            def body(first=None, warm=False):
                # warmup first: the d-init add below waits on the bcol
                # DMA (~4us ring latency) and would stall vector's
                # warm_t memset, delaying the junk chain
                if warm:
                    warmup()
                for ob in range(OB):
                    nc.vector.memset(d_t[ob], 0.0)
                    nc.vector.tensor_scalar_add(out=d_t[ob], in0=d_t[ob],
                                                scalar1=bcol_t[:, ob:ob + 1])
                for p in range(NPASS):
                    if p == 0 and first is not None:
                        xh = first
                    else:
                        xh = xhpool.tile([128, NF16], F16, tag="xh")
                        for c in range(XCH):
                            xsl = slice(c * xsz, (c + 1) * xsz)
                            eng = nc.sync if c % 2 == 0 else nc.scalar
                            eng.dma_start(out=xh[:, xsl],
                                          in_=xh16_d[p][:, xsl])

                    xh_kv = xh.rearrange("p (kt n) -> p kt n", kt=KT)

                    hi_t = [psum_pool.tile([128, 2 * B], F32, tag=f"hi{ob}",
                                           name=f"hi{p}_{ob}")
                            for ob in range(OB)]

                    def mm_hi(ob, kt):
                        osl = slice(ob * 128, (ob + 1) * 128)
                        nc.tensor.matmul(hi_t[ob], wh_k[:, kt, osl],
                                         xh_kv[:, kt, :],
                                         start=(kt == 0), stop=(kt == KT - 1))

                    def drain_ti(ob, ti, last_src=None):
                        osl = slice(ob * 128, (ob + 1) * 128)
                        t = 2 * p + ti
                        bsl = slice(ti * B, (ti + 1) * B)
                        s_sb = spool.tile([128, B], F8, tag="s")
                        m16_sb = opool.tile([128, B], F16, tag="m16")
                        if last_src is not None:
                            # no u-chain after the final step, so the add
                            # can narrow to fp16 directly and its DMA (the
                            # long pole at the kernel tail) issues with no
                            # copy in between, split across both DGE
                            # rings; s thresholds the fp16 m
                            nc.vector.tensor_add(out=m16_sb,
                                                 in0=last_src, in1=d_t[ob])
                            h = B // 2
                            nc.sync.dma_start(out=m_d[t, osl, :h],
                                              in_=m16_sb[:, :h])
                            nc.scalar.dma_start(out=m_d[t, osl, h:],
                                                in_=m16_sb[:, h:])
                            nc.vector.tensor_scalar(
                                out=s_sb, in0=m16_sb, scalar1=M_TH,
                                scalar2=None, op0=mybir.AluOpType.is_ge)
                            nc.sync.dma_start(out=s_d[t, osl, :], in_=s_sb)
                            return
                        m_sb = mpool.tile([128, B], F32, tag="m")
                        nc.vector.tensor_add(out=m_sb,
                                             in0=hi_t[ob][:, bsl],
                                             in1=d_t[ob])
                        nc.vector.tensor_scalar(
                            out=s_sb, in0=m_sb, scalar1=M_TH,
                            scalar2=None, op0=mybir.AluOpType.is_ge)
                        if OFFLOAD:
                            nc.scalar.copy(m16_sb, m_sb)
                        else:
                            nc.vector.tensor_copy(out=m16_sb, in_=m_sb)
                        nc.sync.dma_start(out=m_d[t, osl, :], in_=m16_sb)
                        # s rides the scalar-engine ring: at the tail the
                        # two output DMAs go out in parallel
                        nc.scalar.dma_start(out=s_d[t, osl, :], in_=s_sb)
                        if t < T - 1:  # d is dead after the last step
                            # d = alpha*m*(m < th) + b, fused as two
                            # vector ops -- no scalar-engine ACTIVATE, so
                            # the Act ring carries only DMA traffic
                            u_sb = upool.tile([128, B], F32, tag="u")
                            nc.vector.scalar_tensor_tensor(
                                out=u_sb, in0=m_sb, scalar=M_TH,
                                in1=m_sb,
                                op0=mybir.AluOpType.is_lt,
                                op1=mybir.AluOpType.mult)
                            nc.vector.tensor_scalar(
                                out=d_t[ob], in0=u_sb, scalar1=ALPHA,
                                scalar2=bcol_t[:, ob:ob + 1],
                                op0=mybir.AluOpType.mult,
                                op1=mybir.AluOpType.add)

                    def drain(ob):
                        for ti in range(2):
                            drain_ti(ob, ti)

                    if TISPLIT and p == NPASS - 1:
                        # Last pass: obs 0-2 run full N=512 chains as
                        # usual; only ob3 splits per-t so its t=6 drain
                        # (with the d-state update) overlaps the t=7
                        # matmuls and one short drain remains after the
                        # final matmul. The t=7 chain accumulates in the
                        # retired warmup bank, so the PE never writes
                        # the bank the t=6 drain is reading.
                        LB = OB - 1
                        for ob in range(LB):
                            for kt in range(KT):
                                mm_hi(ob, kt)
                            drain(ob)
                        osl = slice(LB * 128, (LB + 1) * 128)
                        wp7 = psum_pool.tile([128, B], F32, tag="t7",
                                             name="t7psum")
                        for kt in range(KT):
                            nc.tensor.matmul(hi_t[LB][:, :B],
                                             wh_k[:, kt, osl],
                                             xh_kv[:, kt, :B],
                                             start=(kt == 0),
                                             stop=(kt == KT - 1))
                        for kt in range(KT):
                            nc.tensor.matmul(wp7,
                                             wh_k[:, kt, osl],
                                             xh_kv[:, kt, B:],
                                             start=(kt == 0),
                                             stop=(kt == KT - 1))
                        drain_ti(LB, 0)
                        drain_ti(LB, 1, last_src=wp7[:, :])
                        continue

                    if p == 0:
                        # k-outer while the cold DMA streams in, then
                        # ob-sequential so the chains finish staggered
                        # and drains overlap the tail of the pass.
                        KSPLIT = 24
                        for kt in range(KSPLIT):
                            for ob in range(OB):
                                mm_hi(ob, kt)
                        for ob in range(OB):
                            for kt in range(KSPLIT, KT):
                                mm_hi(ob, kt)
                            drain(ob)
                    else:
                        for ob in range(OB):
                            for kt in range(KT):
                                mm_hi(ob, kt)
                            drain(ob)

            if reps == 1:
                body(xh0, warm=True)
            elif os.environ.get("BMU_UNROLL") == "1":
                body(xh0, warm=True)
                for _ in range(reps - 1):
                    body()
            else:
                body(xh0, warm=True)
                with tc.For_i(0, reps - 1, 1):
                    body()

    nc.compile()
    return nc


def _get_nc():
    if "nc" not in _cache:
        _cache["nc"] = _build_kernel()
    return _cache["nc"]


def _prepare_in_maps(x: np.ndarray, W: np.ndarray, b: np.ndarray):
    xT = np.ascontiguousarray(x.transpose(0, 2, 1))  # [T, D_in, B]

    def ptile16(a):  # [T, D, B] -> [NPASS, 128, KT*2*B], [kt][ti][b]
        return np.ascontiguousarray(
            a.reshape(NPASS, 2, KT, 128, B).transpose(0, 3, 2, 1, 4)
            .reshape(NPASS, 128, KT * 2 * B))

    def wtile16(a):  # [D, O] -> [128, KT*O]
        o = a.shape[1]
        return np.ascontiguousarray(
            a.reshape(KT, 128, o).transpose(1, 0, 2).reshape(128, KT * o))

    xh16_t = ptile16(xT.astype(NP_F16))

    in_maps = []
    for c in range(N_CORES):
        sl = slice(c * O_SHARD, (c + 1) * O_SHARD)
        Wt = np.ascontiguousarray(W[sl, :].T)  # [D, O]
        bcol = np.ascontiguousarray(
            b[sl].astype(np.float32).reshape(OB, 128).T)  # [128, OB]
        in_maps.append({
            "wh16": wtile16(Wt.astype(NP_F16)),
            "xh16": xh16_t,
            "bcol": bcol,
        })
    return in_maps


def kernel(x: np.ndarray, W: np.ndarray, b: np.ndarray):
    x = np.asarray(x, dtype=np.float32)
    W = np.asarray(W, dtype=np.float32)
    b = np.asarray(b, dtype=np.float32)
    nc = _get_nc()
    in_maps = _prepare_in_maps(x, W, b)
    res = None
    for attempt in range(3):
        try:
            res = run_bass_kernel_spmd(nc, in_maps,
                                       core_ids=list(range(N_CORES)))
            break
        except Exception:
            # transient device errors (NRT INTERNAL/UNRECOVERABLE) clear
            # on retry; re-raise only if persistent
            if attempt == 2:
                raise
    m = np.empty((T, B, D), dtype=np.float32)
    s = np.empty((T, B, D), dtype=np.float32)
    for c in range(N_CORES):
        sl = slice(c * O_SHARD, (c + 1) * O_SHARD)
        m[:, :, sl] = res.results[c]["m_out"].astype(np.float32) \
            .transpose(0, 2, 1)
        s[:, :, sl] = res.results[c]["s_out"].astype(np.float32) \
            .transpose(0, 2, 1)
    return (m, s)



# revision 32
# speedup vs baseline: 1.0107x; 1.0107x over previous
"""Trainium2 Bass kernel for nn_BoundMemUpdate (spiking membrane update).

Computes, for x:[T,B,D], W:[D,D], b:[D]:
    mm[t] = x[t] @ W.T + b
    m[t] = mm[t] + m[t-1] * (1 - s[t-1]) * 0.5
    s[t] = (m[t] >= 1.0)
Returns (m, s), each [T, B, D] float32.

Sharding: output-dim (D_out) sharded 8 ways across cores (512 each);
x replicated, W/b sharded by rows. The recurrence is per-neuron
elementwise, so no cross-core communication is needed.

Matmul: single-term fp16. x and W are rounded to fp16; the PE forms
exact fp16 products with fp32 accumulation, so the only error is the
input rounding (~2^-11 relative per operand). On the fixed seed-0
problem instance this measures 6.5e-3 concatenated relative error
(506 spike flips of 8.4M) against the fp32 reference -- 3x inside
the 2e-2 gate. W is the PE-stationary operand and x the moving one,
giving output tiles [o_part, (t,b)] and 512 matmuls of 518 cycles
per core (~111 us of PE streaming at 2.4 GHz).

Schedule: 4 passes over t-pairs, 4 o-block PSUM chains per pass with
K=4096 contraction. The vector engine fuses the drain with the
temporal recurrence (scalar_tensor_tensor / tensor_scalar with a
per-partition bias AP -- no scalar-engine ACTIVATE, so the Act ring
carries only DMA traffic and skips its table load). x is stored
pass-interleaved in DRAM ([pass][128][kt][t][b]) so every DMA is
contiguous on both sides. Pass 0 streams k-outer so the PE consumes
(W, x) chunks as they land. The NEFF preamble blocks all DMA for the
first ~7us and the first chunk completion lands ~4.5us after the
queues open, so a 12-matmul junk chain (no DMA dependencies) ramps
the HAM clock gate to 2.4 GHz during exactly that dead window and
hands off to the real k-outer stream as the first (W, x) chunks
arrive; 512 KB chunks keep the early completion cadence (~2.5-3us
per chunk, latency-floored) ahead of warm consumption (3.5us per
4-kt chunk). The last pass splits each o-block chain into per-t
half-bank chains (all t=2p chains before any t=2p+1 chain) so the
expensive t=6 drain overlaps t=7 matmuls and only one short drain
(fp16 add straight into the output tile, m on the SP ring / s on the
Act ring in parallel) remains after the final matmul. Outputs go out
as m:fp16 / s:fp8 in [t, o, b] layout; the host widens and
transposes during the final gather.
"""
import os
import numpy as np

import concourse.bass as bass
import concourse.mybir as mybir
from concourse import bacc
from concourse.tile import TileContext
from concourse.bass_utils import run_bass_kernel_spmd

T, B, D = 8, 256, 4096
N_CORES = 8
O_SHARD = D // N_CORES   # 512
KT = D // 128            # 32 fp16 k-tiles
OB = O_SHARD // 128      # 4 output blocks
NPASS = T // 2           # 4 t-pair passes
ALPHA = 0.5
M_TH = 1.0
WARM_MM = int(os.environ.get("BMU_WARM", "12"))
PSB = int(os.environ.get("BMU_PSB", "1"))       # psum pool bufs
XCH = int(os.environ.get("BMU_XCH", "8"))       # x DMA chunks per pass
WCH = int(os.environ.get("BMU_WCH", "8"))       # W DMA chunks
# pass-0 chunk sizes in kt units (each kt = 512 cols of both W and x).
# 512 KB chunks: early DMA completions land at a latency-floored
# ~2.5-3us cadence regardless of size, so chunks must carry >=4 kt to
# outpace warm PE consumption (0.864us/kt); fewer, bigger chunks also
# keep the issue cost (~0.6us of engine time per dma_start, <=4 in
# flight per ring) low.
CH0 = [int(v) for v in
       os.environ.get("BMU_CH0", "5,4,4,4,4,4,4,3").split(",")]
assert sum(CH0) == KT
OFFLOAD = os.environ.get("BMU_OFFLOAD", "0") == "1"  # s8->gpsimd, m16->scalar
TISPLIT = os.environ.get("BMU_TISPLIT", "1") == "1"  # last pass: per-t chains

F16 = mybir.dt.float16
F8 = mybir.dt.float8e4
F32 = mybir.dt.float32
NP_F16 = np.float16

_cache = {}


def _build_kernel(reps: int = 1):
    nc = bacc.Bacc("TRN2", target_bir_lowering=False, debug=False,
                   num_devices=N_CORES)

    NF16 = KT * 2 * B        # per-pass x free size (16384)

    wh16_d = nc.dram_tensor("wh16", [128, KT * O_SHARD], F16,
                            kind="ExternalInput").ap()
    xh16_d = nc.dram_tensor("xh16", [NPASS, 128, NF16], F16,
                            kind="ExternalInput").ap()
    bcol_d = nc.dram_tensor("bcol", [128, OB], F32,
                            kind="ExternalInput").ap()
    m_d = nc.dram_tensor("m_out", [T, O_SHARD, B], F16,
                         kind="ExternalOutput").ap()
    s_d = nc.dram_tensor("s_out", [T, O_SHARD, B], F8,
                         kind="ExternalOutput").ap()

    with TileContext(nc) as tc:
        with tc.tile_pool(name="wpool", bufs=1) as wpool, \
             tc.tile_pool(name="xhpool", bufs=2) as xhpool, \
             tc.tile_pool(name="cpool", bufs=1) as cpool, \
             tc.tile_pool(name="mpool", bufs=4) as mpool, \
             tc.tile_pool(name="opool", bufs=4) as opool, \
             tc.tile_pool(name="spool", bufs=4) as spool, \
             tc.tile_pool(name="upool", bufs=3) as upool, \
             tc.tile_pool(name="psum", bufs=PSB, space="PSUM") as psum_pool:

            whs = wpool.tile([128, KT * O_SHARD], F16, name="whs")
            bcol_t = cpool.tile([128, OB], F32)
            d_t = [cpool.tile([128, B], F32, name=f"d{ob}") for ob in range(OB)]
            warm_t = cpool.tile([128, 2 * B], F16, name="warm")
            dump_t = cpool.tile([128, 16], F32, name="dump")

            xsz = NF16 // XCH
            wsz = KT * O_SHARD // WCH

            # pass-0 load, interleaved in PE consumption order (k-outer
            # pass 0 walks kt 0..31 across o-blocks, so pair (wh16, xh16)
            # chunks by kt range); the junk warmup chain covers the
            # ~4.5us from queue-open to the first chunk completion.
            xh0 = xhpool.tile([128, NF16], F16, tag="xh")
            # W rides the Activation-engine DGE queue, x the SP queue --
            # two hardware rings drain the cold start in parallel. The
            # first W/x chunks gate the first matmul, so they issue
            # before anything else; bcol (needed only at the first
            # drain, ~15us later) rides behind the first W chunk.
            off = 0
            for ci, ckt in enumerate(CH0):
                sl = slice(off * 512, (off + ckt) * 512)
                nc.scalar.dma_start(out=whs[:, sl], in_=wh16_d[:, sl])
                nc.sync.dma_start(out=xh0[:, sl], in_=xh16_d[0][:, sl])
                if ci == 0:
                    nc.scalar.dma_start(out=bcol_t, in_=bcol_d)
                off += ckt

            wh_k = whs.rearrange("p (kt o) -> p kt o", kt=KT)

            def warmup():
                if WARM_MM <= 0:
                    return
                # vector memset: ready well inside the NEFF preamble, so
                # the junk chain starts the instant the PE queue opens
                nc.vector.memset(warm_t, 0.0)
                # dedicated bank: no real chain ever waits on the
                # warmup drain (5 of 8 PSUM banks in use)
                wp = psum_pool.tile([128, 2 * B], F32, tag="wps",
                                    name="warmpsum")
                for i in range(WARM_MM):
                    nc.tensor.matmul(wp, warm_t[:, :128], warm_t,
                                     start=(i == 0), stop=(i == WARM_MM - 1))
                nc.vector.tensor_copy(out=dump_t, in_=wp[:, :16])

            def body(first=None, warm=False):
                # warmup first: the d-init add below waits on the bcol
                # DMA (~4us ring latency) and would stall vector's
                # warm_t memset, delaying the junk chain
                if warm:
                    warmup()
                for ob in range(OB):
                    nc.vector.memset(d_t[ob], 0.0)
                    nc.vector.tensor_scalar_add(out=d_t[ob], in0=d_t[ob],
                                                scalar1=bcol_t[:, ob:ob + 1])
                for p in range(NPASS):
                    if p == 0 and first is not None:
                        xh = first
                    else:
                        xh = xhpool.tile([128, NF16], F16, tag="xh")
                        for c in range(XCH):
                            xsl = slice(c * xsz, (c + 1) * xsz)
                            eng = nc.sync if c % 2 == 0 else nc.scalar
                            eng.dma_start(out=xh[:, xsl],
                                          in_=xh16_d[p][:, xsl])

                    xh_kv = xh.rearrange("p (kt n) -> p kt n", kt=KT)

                    hi_t = [psum_pool.tile([128, 2 * B], F32, tag=f"hi{ob}",
                                           name=f"hi{p}_{ob}")
                            for ob in range(OB)]

                    def mm_hi(ob, kt):
                        osl = slice(ob * 128, (ob + 1) * 128)
                        nc.tensor.matmul(hi_t[ob], wh_k[:, kt, osl],
                                         xh_kv[:, kt, :],
                                         start=(kt == 0), stop=(kt == KT - 1))

                    def drain_ti(ob, ti, last_src=None):
                        osl = slice(ob * 128, (ob + 1) * 128)
                        t = 2 * p + ti
                        bsl = slice(ti * B, (ti + 1) * B)
                        s_sb = spool.tile([128, B], F8, tag="s")
                        m16_sb = opool.tile([128, B], F16, tag="m16")
                        if last_src is not None:
                            # no u-chain after the final step, so the add
                            # can narrow to fp16 directly and its DMA (the
                            # long pole at the kernel tail) issues with no
                            # copy in between; s thresholds the fp16 m
                            nc.vector.tensor_add(out=m16_sb,
                                                 in0=last_src, in1=d_t[ob])
                            nc.sync.dma_start(out=m_d[t, osl, :], in_=m16_sb)
                            nc.vector.tensor_scalar(
                                out=s_sb, in0=m16_sb, scalar1=M_TH,
                                scalar2=None, op0=mybir.AluOpType.is_ge)
                            nc.scalar.dma_start(out=s_d[t, osl, :], in_=s_sb)
                            return
                        m_sb = mpool.tile([128, B], F32, tag="m")
                        nc.vector.tensor_add(out=m_sb,
                                             in0=hi_t[ob][:, bsl],
                                             in1=d_t[ob])
                        nc.vector.tensor_scalar(
                            out=s_sb, in0=m_sb, scalar1=M_TH,
                            scalar2=None, op0=mybir.AluOpType.is_ge)
                        if OFFLOAD:
                            nc.scalar.copy(m16_sb, m_sb)
                        else:
                            nc.vector.tensor_copy(out=m16_sb, in_=m_sb)
                        nc.sync.dma_start(out=m_d[t, osl, :], in_=m16_sb)
                        # s rides the scalar-engine ring: at the tail the
                        # two output DMAs go out in parallel
                        nc.scalar.dma_start(out=s_d[t, osl, :], in_=s_sb)
                        if t < T - 1:  # d is dead after the last step
                            # d = alpha*m*(m < th) + b, fused as two
                            # vector ops -- no scalar-engine ACTIVATE, so
                            # the Act ring carries only DMA traffic
                            u_sb = upool.tile([128, B], F32, tag="u")
                            nc.vector.scalar_tensor_tensor(
                                out=u_sb, in0=m_sb, scalar=M_TH,
                                in1=m_sb,
                                op0=mybir.AluOpType.is_lt,
                                op1=mybir.AluOpType.mult)
                            nc.vector.tensor_scalar(
                                out=d_t[ob], in0=u_sb, scalar1=ALPHA,
                                scalar2=bcol_t[:, ob:ob + 1],
                                op0=mybir.AluOpType.mult,
                                op1=mybir.AluOpType.add)

                    def drain(ob):
                        for ti in range(2):
                            drain_ti(ob, ti)

                    if TISPLIT and p == NPASS - 1:
                        # Last pass: per-t half-bank chains so the t=6
                        # drain (with its d-state update) overlaps the
                        # t=7 matmuls, and only one short drain remains
                        # after the final matmul. All ti=0 chains run
                        # before any ti=1 chain touches the same bank's
                        # other half, so PE writes never overlap the
                        # vector's reads of the t=6 halves.
                        for ti in range(2):
                            for ob in range(OB):
                                osl = slice(ob * 128, (ob + 1) * 128)
                                bsl = slice(ti * B, (ti + 1) * B)
                                for kt in range(KT):
                                    nc.tensor.matmul(
                                        hi_t[ob][:, bsl],
                                        wh_k[:, kt, osl],
                                        xh_kv[:, kt, bsl],
                                        start=(kt == 0),
                                        stop=(kt == KT - 1))
                                drain_ti(ob, ti,
                                         last_src=(hi_t[ob][:, bsl]
                                                   if (ti == 1 and
                                                       ob == OB - 1)
                                                   else None))
                        continue

                    if p == 0:
                        # k-outer while the cold DMA streams in, then
                        # ob-sequential so the chains finish staggered
                        # and drains overlap the tail of the pass.
                        KSPLIT = 24
                        for kt in range(KSPLIT):
                            for ob in range(OB):
                                mm_hi(ob, kt)
                        for ob in range(OB):
                            for kt in range(KSPLIT, KT):
                                mm_hi(ob, kt)
                            drain(ob)
                    else:
                        for ob in range(OB):
                            for kt in range(KT):
                                mm_hi(ob, kt)
                            drain(ob)

            if reps == 1:
                body(xh0, warm=True)
            elif os.environ.get("BMU_UNROLL") == "1":
                body(xh0, warm=True)
                for _ in range(reps - 1):
                    body()
            else:
                body(xh0, warm=True)
                with tc.For_i(0, reps - 1, 1):
                    body()

    nc.compile()
    return nc


def _get_nc():
    if "nc" not in _cache:
        _cache["nc"] = _build_kernel()
    return _cache["nc"]


def _prepare_in_maps(x: np.ndarray, W: np.ndarray, b: np.ndarray):
    xT = np.ascontiguousarray(x.transpose(0, 2, 1))  # [T, D_in, B]

    def ptile16(a):  # [T, D, B] -> [NPASS, 128, KT*2*B], [kt][ti][b]
        return np.ascontiguousarray(
            a.reshape(NPASS, 2, KT, 128, B).transpose(0, 3, 2, 1, 4)
            .reshape(NPASS, 128, KT * 2 * B))

    def wtile16(a):  # [D, O] -> [128, KT*O]
        o = a.shape[1]
        return np.ascontiguousarray(
            a.reshape(KT, 128, o).transpose(1, 0, 2).reshape(128, KT * o))

    xh16_t = ptile16(xT.astype(NP_F16))

    in_maps = []
    for c in range(N_CORES):
        sl = slice(c * O_SHARD, (c + 1) * O_SHARD)
        Wt = np.ascontiguousarray(W[sl, :].T)  # [D, O]
        bcol = np.ascontiguousarray(
            b[sl].astype(np.float32).reshape(OB, 128).T)  # [128, OB]
        in_maps.append({
            "wh16": wtile16(Wt.astype(NP_F16)),
            "xh16": xh16_t,
            "bcol": bcol,
        })
    return in_maps


def kernel(x: np.ndarray, W: np.ndarray, b: np.ndarray):
    x = np.asarray(x, dtype=np.float32)
    W = np.asarray(W, dtype=np.float32)
    b = np.asarray(b, dtype=np.float32)
    nc = _get_nc()
    in_maps = _prepare_in_maps(x, W, b)
    res = None
    for attempt in range(3):
        try:
            res = run_bass_kernel_spmd(nc, in_maps,
                                       core_ids=list(range(N_CORES)))
            break
        except Exception:
            # transient device errors (NRT INTERNAL/UNRECOVERABLE) clear
            # on retry; re-raise only if persistent
            if attempt == 2:
                raise
    m = np.empty((T, B, D), dtype=np.float32)
    s = np.empty((T, B, D), dtype=np.float32)
    for c in range(N_CORES):
        sl = slice(c * O_SHARD, (c + 1) * O_SHARD)
        m[:, :, sl] = res.results[c]["m_out"].astype(np.float32) \
            .transpose(0, 2, 1)
        s[:, :, sl] = res.results[c]["s_out"].astype(np.float32) \
            .transpose(0, 2, 1)
    return (m, s)

